# revision 1
# baseline (speedup 1.0000x reference)
"""GAT layer (single head, PyG GATConv semantics + relu) on 8 Trainium2 cores.

Strategy (destination-major):
  * Nodes are grouped into 128-node blocks, lexsorted by (deg_lo, deg_hi) so
    every block has near-uniform in-degree. Blocks are dealt round-robin to
    the 8 cores; per-slot grid shapes are equalized across cores (SPMD).
  * Each core builds a full feature table in its HBM: row r = [h bf16 x64 |
    a_src f32 | pad] where h = x@W, a_src = x@(W@att_src). Table rows are
    (node_id + 1); row 0 and row N+1 are pad rows with a_src = -1e4 so that
    padded edge slots contribute exp(...) = 0.
  * Per destination block, incoming-edge source rows are gathered with
    dma_gather (int16 indices => the table is split at row 32768 into a lo
    and a hi table; each block's edge slots are split into a lo column range
    and a hi column range).
  * Softmax (without max-subtraction; logits are O(10) so exp is safe) and
    the weighted sum are computed with per-partition-scalar ops and free-dim
    reductions only; output = relu(sum/denom + bias).
"""

import ml_dtypes
import numpy as np

import concourse.bass as bass
import concourse.tile as tile
from concourse import bacc, mybir
from concourse.bass_utils import run_bass_kernel_spmd

P = 128
NCORES = 8
NEG_SLOPE = 0.2
EPS = 1e-16
PAD_ASRC = -1.0e4


def _ceil_to(x, m):
    return (x + m - 1) // m * m


def _preprocess(edge_index, n_nodes, lo_rows):
    """All host-side index work: blocks, grids, gather index tiles."""
    src = np.asarray(edge_index[0], dtype=np.int64)
    dst = np.asarray(edge_index[1], dtype=np.int64)
    loop = np.arange(n_nodes, dtype=np.int64)
    src2 = np.concatenate([src, loop])
    dst2 = np.concatenate([dst, loop])
    st = src2 + 1  # table rows
    is_hi = st >= lo_rows

    deg = np.bincount(dst2, minlength=n_nodes)
    deg_lo = np.bincount(dst2[~is_hi], minlength=n_nodes)
    deg_hi = deg - deg_lo

    # node blocks: lexsort descending by (deg_lo, deg_hi)
    order = np.lexsort((deg_hi, deg_lo))[::-1].copy()
    nblk_out = _ceil_to(n_nodes, P) // P
    slots = _ceil_to(nblk_out, NCORES) // NCORES
    node_at = np.full((slots * NCORES, P), -1, dtype=np.int64)
    node_at.reshape(-1)[: n_nodes] = order
    # block b -> (core, slot)
    # per-block max degrees
    nd = node_at  # [NBLKS, P]
    valid = nd >= 0
    blk_deg_lo = np.where(valid, deg_lo[np.clip(nd, 0, None)], 0).max(axis=1)
    blk_deg_hi = np.where(valid, deg_hi[np.clip(nd, 0, None)], 0).max(axis=1)
    d_lo = blk_deg_lo.reshape(slots, NCORES).max(axis=1)  # per slot
    d_hi = blk_deg_hi.reshape(slots, NCORES).max(axis=1)

    # node -> (core, slot, p)
    pos = np.full(n_nodes, -1, dtype=np.int64)
    pos[order] = np.arange(n_nodes)
    b_of = pos // P
    p_of = pos % P
    core_of = b_of % NCORES
    slot_of = b_of // NCORES

    # rank of each edge within its destination node, lo-first
    eo = np.lexsort((is_hi, dst2))
    dsts = dst2[eo]
    sts = st[eo]
    his = is_hi[eo]
    off = np.zeros(n_nodes + 1, dtype=np.int64)
    np.cumsum(deg, out=off[1:])
    jj = np.arange(len(eo), dtype=np.int64) - off[dsts]
    jhi = jj - deg_lo[dsts]

    col_off_lo = np.zeros(slots + 1, dtype=np.int64)
    np.cumsum(d_lo, out=col_off_lo[1:])
    col_off_hi = np.zeros(slots + 1, dtype=np.int64)
    np.cumsum(d_hi, out=col_off_hi[1:])
    tot_lo = int(col_off_lo[-1])
    tot_hi = int(col_off_hi[-1])

    padhi_loc = n_nodes + 1 - lo_rows
    glo = np.zeros((NCORES, P, tot_lo), dtype=np.int64)  # pad -> lo row 0
    ghi = np.full((NCORES, P, tot_hi), padhi_loc, dtype=np.int64)

    ek = core_of[dsts]
    ei_slot = slot_of[dsts]
    ep = p_of[dsts]
    for k in range(NCORES):
        ml = (ek == k) & ~his
        glo[k][ep[ml], col_off_lo[ei_slot[ml]] + jj[ml]] = sts[ml]
        mh = (ek == k) & his
        ghi[k][ep[mh], col_off_hi[ei_slot[mh]] + jhi[mh]] = sts[mh] - lo_rows

    return dict(
        d_lo=d_lo, d_hi=d_hi, col_off_lo=col_off_lo, col_off_hi=col_off_hi,
        glo=glo, ghi=ghi, node_at=node_at, slots=slots,
    )


def _make_superchunks(d_lo, d_hi, cmax):
    """Group consecutive slots into super-chunks with <= cmax total columns.

    The first 2 and last 3 slots go in single-slot chunks so the pipeline
    ramps up quickly and the post-last-gather drain chain is short."""
    n = len(d_lo)
    scs = []
    cur = []
    cur_c = 0
    for i in range(n):
        c = int(d_lo[i] + d_hi[i])
        single = i < 2 or i >= n - 3
        if cur and (single or cur_c + c > cmax):
            scs.append(cur)
            cur = []
            cur_c = 0
        cur.append(i)
        cur_c += c
        if single:
            scs.append(cur)
            cur = []
            cur_c = 0
    if cur:
        scs.append(cur)
    return scs


def _wrap_idx(arr):
    """dma_gather index layout: [128, n/16] int16, idx i at (i%16, i//16),
    replicated across the 8 Q7 core groups."""
    n = arr.shape[0]
    assert n % 16 == 0
    w = arr.reshape(n // 16, 16).T.astype(np.int16)  # [16, n/16]
    return np.tile(w, (8, 1))


def _build_gidx(meta, scs):
    """Concatenate per-call wrapped index tiles; record call metadata."""
    d_lo, d_hi = meta["d_lo"], meta["d_hi"]
    col_off_lo, col_off_hi = meta["col_off_lo"], meta["col_off_hi"]
    calls = []  # per sc: (clo, chi, off16_lo, len16_lo, off16_hi, len16_hi)
    gidx = [[] for _ in range(NCORES)]
    off16 = 0
    for sc in scs:
        i0, i1 = sc[0], sc[-1] + 1
        a0, a1 = int(col_off_lo[i0]), int(col_off_lo[i1])
        b0, b1 = int(col_off_hi[i0]), int(col_off_hi[i1])
        clo, chi = a1 - a0, b1 - b0
        lo_len16 = clo * P // 16
        hi_len16 = chi * P // 16
        for k in range(NCORES):
            lo_list = meta["glo"][k][:, a0:a1].T.ravel()
            hi_list = meta["ghi"][k][:, b0:b1].T.ravel()
            gidx[k].append(_wrap_idx(lo_list))
            gidx[k].append(_wrap_idx(hi_list))
        calls.append((clo, chi, off16, lo_len16, off16 + lo_len16, hi_len16))
        off16 += lo_len16 + hi_len16
    gidx = [np.concatenate(g, axis=1) if g else np.zeros((P, 0), np.int16)
            for g in gidx]
    return gidx, calls, off16


def _build_nc(cfg, dbg=False):
    trows, lo_rows = cfg["trows"], cfg["lo_rows"]
    slots, scs, calls = cfg["slots"], cfg["scs"], cfg["calls"]
    d_lo, d_hi = cfg["d_lo"], cfg["d_hi"]
    col_off_lo, col_off_hi = cfg["col_off_lo"], cfg["col_off_hi"]
    gc16 = cfg["gc16"]
    f_out = cfg["f_out"]
    nblk_tbl = trows // P
    hi_rows = trows - lo_rows
    wcols = f_out + 2  # W | w_src | w_dst

    nc = bacc.Bacc("TRN2", target_bir_lowering=False, debug=False,
                   num_devices=NCORES, num_swdge_queues=4)
    xTb = nc.dram_tensor("xTb", [P, trows], mybir.dt.bfloat16, kind="ExternalInput")
    wextb = nc.dram_tensor("wextb", [P, wcols], mybir.dt.bfloat16,
                           kind="ExternalInput")
    ownxt = nc.dram_tensor("ownxt", [P, slots * P], mybir.dt.bfloat16,
                           kind="ExternalInput")
    gidx_d = nc.dram_tensor("gidx", [P, max(gc16, 16)], mybir.dt.int16,
                            kind="ExternalInput")
    biasb = nc.dram_tensor("biasb", [P, f_out], mybir.dt.float32,
                           kind="ExternalInput")
    padrow = nc.dram_tensor("padrow", [1, P], mybir.dt.bfloat16,
                            kind="ExternalInput")
    out_d = nc.dram_tensor("out", [slots * P, f_out], mybir.dt.float32,
                           kind="ExternalOutput")
    tbl_lo = nc.dram_tensor("tbl_lo", [lo_rows, P], mybir.dt.bfloat16,
                            kind="Internal")
    tbl_hi = nc.dram_tensor("tbl_hi", [max(hi_rows, P), P], mybir.dt.bfloat16,
                            kind="Internal")
    if dbg:
        csc0 = cfg["calls"][0][0] + cfg["calls"][0][1]
        nb0 = len(cfg["scs"][0])
        dbg_tbl = nc.dram_tensor("dbg_tbl", [lo_rows, P], mybir.dt.bfloat16,
                                 kind="ExternalOutput")
        dbg_adst = nc.dram_tensor("dbg_adst", [P, slots], mybir.dt.float32,
                                  kind="ExternalOutput")
        dbg_g = nc.dram_tensor("dbg_g", [P, csc0 * P], mybir.dt.bfloat16,
                               kind="ExternalOutput")
        dbg_s = nc.dram_tensor("dbg_s", [P, csc0], mybir.dt.float32,
                               kind="ExternalOutput")
        dbg_dn = nc.dram_tensor("dbg_dn", [P, 2 * nb0], mybir.dt.float32,
                                kind="ExternalOutput")

    fp32 = mybir.dt.float32
    bf16 = mybir.dt.bfloat16

    with tile.TileContext(nc) as tc:
        with (
            tc.tile_pool(name="const", bufs=1) as cpool,
            tc.tile_pool(name="xt", bufs=3) as xtpool,
            tc.tile_pool(name="ps", bufs=4, space="PSUM") as pspool,
            tc.tile_pool(name="tstage", bufs=3) as tspool,
            tc.tile_pool(name="gat", bufs=6) as gpool,
            tc.tile_pool(name="sc", bufs=2) as scpool,
            tc.tile_pool(name="blk", bufs=4) as bpool,
        ):
            wextb_sb = cpool.tile([P, wcols], bf16)
            nc.sync.dma_start(out=wextb_sb[:], in_=wextb[:])
            biasb_sb = cpool.tile([P, f_out], fp32)
            nc.sync.dma_start(out=biasb_sb[:], in_=biasb[:])
            ownxt_sb = cpool.tile([P, slots * P], bf16)
            nc.sync.dma_start(out=ownxt_sb[:], in_=ownxt[:])
            gidx_sb = cpool.tile([P, max(gc16, 16)], mybir.dt.int16)
            nc.sync.dma_start(out=gidx_sb[:], in_=gidx_d[:])
            adst_own = cpool.tile([P, slots], fp32)
            adst02 = cpool.tile([P, slots], fp32)
            epst = cpool.tile([P, slots], fp32)
            nc.vector.memset(epst[:], EPS)
            adst08 = cpool.tile([P, slots], fp32)

            # ---- phase A: build the table ----
            WB = 8  # blocks per load / table-write batch
            for g0 in range(0, nblk_tbl, WB):
                gn = min(WB, nblk_tbl - g0)
                tstage = tspool.tile([P, WB, f_out + 4], bf16)
                xtb8 = xtpool.tile([P, WB, P], bf16, tag="xtb")
                nc.sync.dma_start(
                    out=xtb8[:, 0:gn, :],
                    in_=xTb[:, g0 * P:(g0 + gn) * P].rearrange(
                        "p (i q) -> p i q", q=P))
                for bi in range(gn):
                    ps = pspool.tile([P, f_out + 2], fp32, tag="psh")
                    nc.tensor.matmul(out=ps[:], lhsT=xtb8[:, bi, :].squeeze(),
                                     rhs=wextb_sb[:, 0:f_out + 2],
                                     start=True, stop=True)
                    if bi % 2 == 0:
                        nc.scalar.copy(out=tstage[:, bi, 0:f_out],
                                       in_=ps[:, 0:f_out])
                    else:
                        nc.vector.tensor_copy(out=tstage[:, bi, 0:f_out],
                                              in_=ps[:, 0:f_out])
                    nc.vector.tensor_copy(
                        out=tstage[:, bi, f_out:f_out + 4].bitcast(fp32),
                        in_=ps[:, f_out:f_out + 2])
                r0 = g0 * P
                r1 = r0 + gn * P
                if r1 <= lo_rows:
                    dst_ap = tbl_lo[r0:r1, 0:f_out + 4]
                else:
                    assert r0 >= lo_rows
                    dst_ap = tbl_hi[r0 - lo_rows:r1 - lo_rows, 0:f_out + 4]
                nc.sync.dma_start(
                    out=dst_ap.rearrange("(i p) w -> p i w", p=P),
                    in_=tstage[:, 0:gn, :])
                if g0 == 0:
                    # overwrite row 0 with the pad row as soon as possible
                    nc.sync.dma_start(out=tbl_lo[0:1, :], in_=padrow[:])
            if hi_rows > 0:
                ph = cfg["n_nodes"] + 1 - lo_rows
                nc.sync.dma_start(out=tbl_hi[ph:ph + 1, :], in_=padrow[:])

            # a_dst for own nodes
            for i in range(slots):
                ps2 = pspool.tile([P, 1], fp32, tag="adst")
                nc.tensor.matmul(out=ps2[:], lhsT=ownxt_sb[:, i * P:(i + 1) * P],
                                 rhs=wextb_sb[:, f_out + 1:f_out + 2],
                                 start=True, stop=True)
                nc.vector.tensor_copy(out=adst_own[:, i:i + 1], in_=ps2[:])
            nc.vector.tensor_scalar_mul(adst02[:], adst_own[:], NEG_SLOPE)

            # ---- phase B: gather + softmax + weighted sum ----
            q = 0
            for sci, sc in enumerate(scs):
                clo, chi, off_lo, len_lo, off_hi, len_hi = calls[sci]
                csc = clo + chi
                nb = len(sc)
                g_t = gpool.tile([P, csc, P], bf16)
                if clo > 0:
                    nc.gpsimd.dma_gather(
                        out_ap=g_t[:, 0:clo, :], in_ap=tbl_lo[:],
                        idxs_ap=gidx_sb[:, off_lo:off_lo + len_lo],
                        num_idxs=clo * P, num_idxs_reg=clo * P,
                        elem_size=P, single_packet=False, queue_num=sci % 4)
                if chi > 0:
                    nc.gpsimd.dma_gather(
                        out_ap=g_t[:, clo:csc, :], in_ap=tbl_hi[:],
                        idxs_ap=gidx_sb[:, off_hi:off_hi + len_hi],
                        num_idxs=chi * P, num_idxs_reg=chi * P,
                        elem_size=P, single_packet=False, queue_num=(sci + 2) % 4)

                # s = exp(lrelu(z)) = max(exp(z), exp(0.2 z))
                s_t = scpool.tile([P, csc], fp32, tag="s")
                e1_t = scpool.tile([P, csc], fp32, tag="e1")
                e3_t = scpool.tile([P, csc], fp32, tag="e3")
                dn_t = scpool.tile([P, 2 * nb], fp32, tag="dn")
                i0 = sc[0]
                for bi, i in enumerate(sc):
                    for half, (h0, h1) in enumerate([
                        (int(cfg["col_off_lo"][i] - cfg["col_off_lo"][i0]),
                         int(cfg["col_off_lo"][i + 1] - cfg["col_off_lo"][i0])),
                        (clo + int(cfg["col_off_hi"][i] - cfg["col_off_hi"][i0]),
                         clo + int(cfg["col_off_hi"][i + 1] - cfg["col_off_hi"][i0])),
                    ]):
                        dslice = dn_t[:, 2 * bi + half:2 * bi + half + 1]
                        if h1 == h0:
                            nc.vector.memset(dslice, 0.0)
                            continue
                        asrcv = g_t[:, h0:h1, f_out:f_out + 2].bitcast(fp32)
                        nc.scalar.activation(
                            out=e1_t[:, h0:h1], in_=asrcv.squeeze(),
                            func=mybir.ActivationFunctionType.Exp,
                            bias=adst_own[:, i:i + 1], scale=1.0)
                        nc.scalar.activation(
                            out=e3_t[:, h0:h1], in_=asrcv.squeeze(),
                            func=mybir.ActivationFunctionType.Exp,
                            bias=adst02[:, i:i + 1], scale=NEG_SLOPE)
                        nc.vector.tensor_tensor(
                            out=s_t[:, h0:h1], in0=e1_t[:, h0:h1],
                            in1=e3_t[:, h0:h1], op=mybir.AluOpType.max)
                        nc.vector.tensor_reduce(
                            out=dslice, in_=s_t[:, h0:h1],
                            axis=mybir.AxisListType.X,
                            op=mybir.AluOpType.add)

                if dbg and sci == 0:
                    nc.sync.dma_start(out=dbg_g[:],
                                      in_=g_t[:].rearrange("p c w -> p (c w)"))
                    nc.sync.dma_start(out=dbg_s[:], in_=s_t[:])
                    nc.sync.dma_start(out=dbg_dn[:], in_=dn_t[:])

                dsum = bpool.tile([P, nb], fp32, tag="dsum")
                nc.vector.tensor_reduce(
                    out=dsum[:],
                    in_=dn_t[:].rearrange("p (b t) -> p b t", t=2),
                    axis=mybir.AxisListType.X,
                    op=mybir.AluOpType.add)
                rec = bpool.tile([P, nb], fp32, tag="rec")
                nc.vector.tensor_add(dsum[:], dsum[:], epst[:, 0:nb])
                nc.vector.reciprocal(rec[:], dsum[:])

                s16 = scpool.tile([P, csc], bf16, tag="s16")
                nc.scalar.copy(out=s16[:], in_=s_t[:])
                wgt = scpool.tile([P, csc, f_out], bf16, tag="wgt")
                nc.vector.tensor_tensor(
                    out=wgt[:], in0=g_t[:, :, 0:f_out],
                    in1=s16[:].unsqueeze(2).broadcast_to([P, csc, f_out]),
                    op=mybir.AluOpType.mult)

                t1a = bpool.tile([P, nb, f_out], fp32, tag="t1a")
                t2a = bpool.tile([P, nb, f_out], fp32, tag="t2a")
                ostage = scpool.tile([P, nb, f_out], fp32, tag="ostage")
                for bi, i in enumerate(sc):
                    a0 = int(cfg["col_off_lo"][i] - cfg["col_off_lo"][i0])
                    a1 = int(cfg["col_off_lo"][i + 1] - cfg["col_off_lo"][i0])
                    b0 = clo + int(cfg["col_off_hi"][i] - cfg["col_off_hi"][i0])
                    b1 = clo + int(cfg["col_off_hi"][i + 1] - cfg["col_off_hi"][i0])
                    if a1 > a0:
                        nc.vector.tensor_reduce(
                            out=t1a[:, bi, :],
                            in_=wgt[:, a0:a1, :].rearrange("p c f -> p f c"),
                            axis=mybir.AxisListType.X, op=mybir.AluOpType.add)
                    else:
                        nc.vector.memset(t1a[:, bi, :], 0.0)
                    if b1 > b0:
                        nc.vector.tensor_reduce(
                            out=t2a[:, bi, :],
                            in_=wgt[:, b0:b1, :].rearrange("p c f -> p f c"),
                            axis=mybir.AxisListType.X, op=mybir.AluOpType.add)
                    else:
                        nc.vector.memset(t2a[:, bi, :], 0.0)
                nc.vector.tensor_add(t1a[:], t1a[:], t2a[:])
                nc.vector.tensor_tensor(
                    out=t1a[:], in0=t1a[:],
                    in1=rec[:].unsqueeze(2).broadcast_to([P, nb, f_out]),
                    op=mybir.AluOpType.mult)
                nc.vector.tensor_tensor(
                    out=t1a[:], in0=t1a[:],
                    in1=biasb_sb[:].unsqueeze(1).broadcast_to([P, nb, f_out]),
                    op=mybir.AluOpType.add)
                nc.scalar.activation(out=ostage[:], in_=t1a[:],
                                     func=mybir.ActivationFunctionType.Relu)
                nc.sync.dma_start(
                    out=out_d[i0 * P:(i0 + nb) * P, :].rearrange(
                        "(i p) f -> p i f", p=P),
                    in_=ostage[:])
            if dbg:
                nc.sync.dma_start(out=dbg_tbl[:], in_=tbl_lo[:])
                nc.sync.dma_start(out=dbg_adst[:], in_=adst_own[:])
    nc.compile()
    return nc


def _gat_kernel(x, edge_index, W, att_src, att_dst, bias, lo_rows=32768,
                cmax=104, dbg=False):
    n_nodes, f_in = x.shape
    f_out = W.shape[1]
    assert f_in == P
    trows = _ceil_to(n_nodes + 2, P)
    lo_rows = min(lo_rows, trows)

    meta = _preprocess(edge_index, n_nodes, lo_rows)
    scs = _make_superchunks(meta["d_lo"], meta["d_hi"], cmax)
    gidx, calls, gc16 = _build_gidx(meta, scs)

    cfg = dict(trows=trows, lo_rows=lo_rows, slots=meta["slots"], scs=scs,
               calls=calls, d_lo=meta["d_lo"], d_hi=meta["d_hi"],
               col_off_lo=meta["col_off_lo"], col_off_hi=meta["col_off_hi"],
               gc16=gc16, f_out=f_out, n_nodes=n_nodes)
    nc = _build_nc(cfg, dbg=dbg)
    _LAST_META[0] = (meta, cfg)

    # ---- inputs ----
    x = np.asarray(x, dtype=np.float32)
    W = np.asarray(W, dtype=np.float32)
    att_src = np.asarray(att_src, dtype=np.float32)
    att_dst = np.asarray(att_dst, dtype=np.float32)
    bias = np.asarray(bias, dtype=np.float32)

    xT = np.zeros((P, trows), dtype=np.float32)
    xT[:, 1:1 + n_nodes] = x.T
    wext = np.zeros((P, f_out + 2), dtype=np.float32)
    wext[:, 0:f_out] = W
    wext[:, f_out] = W @ att_src
    wext[:, f_out + 1] = W @ att_dst
    xTb = xT.astype(ml_dtypes.bfloat16)
    wextb = wext.astype(ml_dtypes.bfloat16)
    biasb = np.tile(bias[None, :], (P, 1)).astype(np.float32)
    # table row = P bf16 cols: [h x f_out | a_src f32 as 2 cols | pad]
    padrow_f32 = np.zeros(P // 2, dtype=np.float32)
    padrow_f32[f_out // 2] = PAD_ASRC  # f32 word 32 == bf16 cols 64..65
    padrow = padrow_f32.view(ml_dtypes.bfloat16).reshape(1, P).copy()

    in_maps = []
    for k in range(NCORES):
        ox = np.zeros((P, meta["slots"] * P), dtype=np.float32)
        nd = meta["node_at"][k::NCORES].reshape(-1)  # blocks k, k+8,... -> slots
        m = nd >= 0
        ox[:, m] = x[nd[m]].T
        gi = gidx[k]
        if gi.shape[1] < max(gc16, 16):
            gi = np.concatenate(
                [gi, np.zeros((P, max(gc16, 16) - gi.shape[1]), np.int16)], axis=1)
        in_maps.append({
            "xTb": xTb, "wextb": wextb,
            "ownxt": ox.astype(ml_dtypes.bfloat16),
            "gidx": np.ascontiguousarray(gi),
            "biasb": biasb,
            "padrow": padrow,
        })

    res = run_bass_kernel_spmd(nc, in_maps, core_ids=list(range(NCORES)),
                               **_RUN_KW)
    _LAST_RESULT[0] = res

    out = np.zeros((n_nodes, f_out), dtype=np.float32)
    for k in range(NCORES):
        nd = meta["node_at"][k::NCORES].reshape(-1)
        m = nd >= 0
        out[nd[m]] = res.results[k]["out"][m]
    return out


_RUN_KW = {}
_LAST_RESULT = [None]
_LAST_META = [None]


def kernel(x, edge_index, W, att_src, att_dst, bias):
    return _gat_kernel(x, edge_index, W, att_src, att_dst, bias, cmax=48)



# revision 6
# speedup vs baseline: 1.0890x; 1.0890x over previous
"""GAT layer (single head, PyG GATConv semantics + relu) on 8 Trainium2 cores.

Strategy (destination-major, descriptor-minimized):
  * ONE feature table in HBM per core (replicated build): row r = 256B =
    [h bf16 x64 | a_src f32 | pad], rows ordered by FIRST USE so each
    superchunk's gathers only need a table PREFIX -> the table build (phase
    A) overlaps the edge gathers (phase B).  Rows 0 and MID=17408 are pad
    rows (a_src = -1e4 => exp() = 0 for padded edge slots).
  * Two gather windows (int16 indices span 32768 rows): A = rows [0, 32768),
    B = rows [17408, 50176).  Rows in the overlap may be fetched by either
    call; per-edge assignment balances each destination's A/B column counts
    (near-zero split padding).  Each window's columns are further split into
    ~14-col calls spread over the 4 SWDGE queues.
  * Destinations sorted by (dA, dB) (fixed-point), dealt node-round-robin to
    cores so per-slot shapes are SPMD-uniform and tight.
  * Self-loops are folded in locally from ownxt (never gathered).
  * Per-slot compute: exp via per-partition-bias activation is replaced by
    superchunk-fused ops (z-add, 2 exps, max, cast, weighted multiply) with
    only the segment reductions per slot.
"""

import ml_dtypes
import numpy as np

import concourse.bass as bass
import concourse.tile as tile
from concourse import bacc, mybir
from concourse.bass_utils import run_bass_kernel_spmd

P = 128
NCORES = 8
TROWS = 50176
MID = 17408
WINB = 32768
NEG_SLOPE = 0.2
PAD_ASRC = -1.0e4
F_OUT = 64
WCOLS = F_OUT + 2


# --------------------------------------------------------------------------
# host-side preprocessing
# --------------------------------------------------------------------------

def _preprocess(edge_index, n_nodes, n_iter=6, cmax=56):
    src = np.asarray(edge_index[0], dtype=np.int64)
    dst = np.asarray(edge_index[1], dtype=np.int64)
    deg = np.bincount(dst, minlength=n_nodes)
    slots = (n_nodes + P * NCORES - 1) // (P * NCORES)

    order = np.argsort(-deg, kind="stable")
    for it in range(n_iter):
        pos = np.empty(n_nodes, np.int64)
        pos[order] = np.arange(n_nodes)
        slot_of = pos // (P * NCORES)
        e_slot = slot_of[dst]
        first = np.full(n_nodes, slots, np.int64)
        np.minimum.at(first, src, e_slot)
        used = first < slots
        uorder = np.argsort(first[used], kind="stable")
        un = used.nonzero()[0][uorder]
        rank = np.full(n_nodes, -1, np.int64)
        rank[un] = np.arange(len(un))
        pos_t = np.where(rank < MID - 1, rank + 1, rank + 2)
        r = pos_t[src]
        onlyA = r < MID + 1
        onlyB = r >= WINB
        nA = np.bincount(dst[onlyA], minlength=n_nodes)
        nB = np.bincount(dst[onlyB], minlength=n_nodes)
        nM = deg - nA - nB
        tgt = (deg + 1) // 2
        dA = np.clip(tgt, nA, nA + nM)
        dB = deg - dA
        if it < n_iter - 1:
            order = np.lexsort((dB, dA))[::-1].copy()

    cap = slots * NCORES * P
    na = np.full(cap, -1, np.int64)
    na[:n_nodes] = order
    # sorted node r -> core r%8, slot r//1024, p (r//8)%128
    node_at = np.transpose(na.reshape(slots, P, NCORES), (2, 0, 1)).copy()

    da = np.zeros(cap, np.int64)
    db = np.zeros(cap, np.int64)
    da[:n_nodes] = dA[order]
    db[:n_nodes] = dB[order]
    D_A = da.reshape(slots, -1).max(axis=1)
    D_B = db.reshape(slots, -1).max(axis=1)

    prefA = np.full(slots, 2, np.int64)
    prefB = np.full(slots, MID + 2, np.int64)
    mA = r < WINB
    np.maximum.at(prefA, e_slot[mA], r[mA] + 1)
    mB = r >= MID + 1
    np.maximum.at(prefB, e_slot[mB], r[mB] + 1)
    prefA = np.maximum.accumulate(np.minimum(prefA, WINB))
    prefB = np.maximum.accumulate(prefB)

    scs = []
    cur, cur_c = [], 0
    for s in range(slots):
        c = int(D_A[s] + D_B[s])
        single = s < 2 or s >= slots - 2
        if cur and (single or cur_c + c > cmax):
            scs.append(cur)
            cur, cur_c = [], 0
        cur.append(s)
        cur_c += c
        if single:
            scs.append(cur)
            cur, cur_c = [], 0
    if cur:
        scs.append(cur)

    return dict(
        deg=deg, dA=dA, dB=dB, order=order, pos=pos, pos_t=pos_t, rank=rank,
        node_at=node_at, slots=slots, D_A=D_A, D_B=D_B,
        prefA=prefA, prefB=prefB, scs=scs, src=src, dst=dst,
        n_nodes=n_nodes,
    )


def _build_gather_lists(meta):
    """Per (core, sc): (gA [P, colsA], gB [P, colsB]) window-local rows."""
    src, dst, pos = meta["src"], meta["dst"], meta["pos"]
    pos_t = meta["pos_t"]
    dA = meta["dA"]
    D_A, D_B = meta["D_A"], meta["D_B"]
    slots = meta["slots"]
    n_nodes = meta["n_nodes"]

    r_node = pos[dst]
    core_of = r_node % NCORES
    sp = r_node // NCORES
    slot_of = sp // P
    p_of = sp % P

    r = pos_t[src]
    onlyB = r >= WINB
    onlyA = r < MID + 1
    midm = ~onlyA & ~onlyB

    eo = np.lexsort((np.arange(len(src)), dst))
    is_mid = midm[eo]
    dsts = dst[eo]
    midrank = np.zeros(len(eo), np.int64)
    key = dsts[is_mid]
    grp_start = np.zeros(n_nodes + 1, np.int64)
    np.add.at(grp_start[1:], key, 1)
    np.cumsum(grp_start, out=grp_start)
    midrank[is_mid] = np.arange(is_mid.sum()) - grp_start[key]
    nA_map = np.bincount(dst[onlyA], minlength=n_nodes)
    quota = dA - nA_map
    toA = np.zeros(len(eo), bool)
    toA[~is_mid] = onlyA[eo][~is_mid]
    toA[is_mid] = midrank[is_mid] < quota[key]

    ek = core_of[eo]
    es = slot_of[eo]
    ep = p_of[eo]
    er = r[eo]
    skey = dsts * 2 + (~toA).astype(np.int64)
    sord = np.lexsort((np.arange(len(eo)), skey))
    _, first_idx, counts = np.unique(skey[sord], return_index=True,
                                     return_counts=True)
    jj = np.empty(len(eo), np.int64)
    jj[sord] = np.arange(len(eo)) - np.repeat(first_idx, counts)

    sc_of_slot = np.empty(slots, np.int64)
    bi_of_slot = np.empty(slots, np.int64)
    for ci, sc in enumerate(meta["scs"]):
        for bi, s in enumerate(sc):
            sc_of_slot[s] = ci
            bi_of_slot[s] = bi
    sc_offs = []
    for sc in meta["scs"]:
        offA = np.concatenate([[0], np.cumsum(D_A[sc])])
        offB = np.concatenate([[0], np.cumsum(D_B[sc])])
        sc_offs.append((offA.astype(int), offB.astype(int)))

    eci = sc_of_slot[es]
    ebi = bi_of_slot[es]
    lists = {}
    for ci, sc in enumerate(meta["scs"]):
        offA, offB = sc_offs[ci]
        for k in range(NCORES):
            lists[(k, ci)] = (np.zeros((P, int(offA[-1])), np.int64),
                              np.zeros((P, int(offB[-1])), np.int64))
    colA_e = np.zeros(len(eo), np.int64)
    colB_e = np.zeros(len(eo), np.int64)
    for ci in range(len(meta["scs"])):
        offA, offB = sc_offs[ci]
        m = eci == ci
        colA_e[m] = offA[ebi[m]] + jj[m]
        colB_e[m] = offB[ebi[m]] + jj[m]
    for k in range(NCORES):
        mk = ek == k
        ma = mk & toA
        mb = mk & ~toA
        for ci in range(len(meta["scs"])):
            gA, gB = lists[(k, ci)]
            mm = ma & (eci == ci)
            gA[ep[mm], colA_e[mm]] = er[mm]
            mm = mb & (eci == ci)
            gB[ep[mm], colB_e[mm]] = er[mm] - MID
    return lists, sc_offs


def _wrap_idx(arr):
    """dma_gather index layout: [128, n/16] int16, idx i at (i%16, i//16),
    replicated across the 8 Q7 core groups."""
    n = arr.shape[0]
    assert n % 16 == 0
    w = arr.reshape(n // 16, 16).T.astype(np.int16)
    return np.tile(w, (8, 1))


def _plan_calls(meta, lists, chunk_cols=14):
    """Split each sc's A/B column ranges into calls, balance over 4 queues.

    Returns: calls (list of dicts), gidx per core [P, gc16]."""
    calls = []
    qload = [0, 0, 0, 0]
    off16 = 0
    gidx = [[] for _ in range(NCORES)]
    for ci, sc in enumerate(meta["scs"]):
        gA0, gB0 = lists[(0, ci)]
        for side, cols, pref in (
            (0, gA0.shape[1], int(meta["prefA"][sc[-1]])),
            (1, gB0.shape[1], int(meta["prefB"][sc[-1]])),
        ):
            if cols == 0:
                continue
            nch = max(1, -(-cols // chunk_cols))
            bounds = np.linspace(0, cols, nch + 1).astype(int)
            for c0, c1 in zip(bounds[:-1], bounds[1:]):
                if c1 == c0:
                    continue
                q = min(range(4), key=lambda i: qload[i])
                qload[q] += c1 - c0
                ln16 = (c1 - c0) * P // 16
                for k in range(NCORES):
                    g = lists[(k, ci)][side]
                    gidx[k].append(_wrap_idx(g[:, c0:c1].T.ravel()))
                calls.append(dict(ci=ci, side=side, c0=int(c0), c1=int(c1),
                                  pref=pref, q=q, off16=off16, ln16=ln16))
                off16 += ln16
    gidx = [np.concatenate(g, axis=1) if g else np.zeros((P, 16), np.int16)
            for g in gidx]
    return calls, gidx, off16


# --------------------------------------------------------------------------
# device program
# --------------------------------------------------------------------------

def _build_nc(cfg):
    slots = cfg["slots"]
    scs = cfg["scs"]
    sc_offs = cfg["sc_offs"]
    D_A, D_B = cfg["D_A"], cfg["D_B"]
    calls = cfg["calls"]
    gc16 = max(cfg["gc16"], 16)

    fp32 = mybir.dt.float32
    bf16 = mybir.dt.bfloat16

    nc = bacc.Bacc("TRN2", target_bir_lowering=False, debug=False,
                   num_devices=NCORES, num_swdge_queues=4)
    xTb = nc.dram_tensor("xTb", [P, TROWS], bf16, kind="ExternalInput")
    wextb = nc.dram_tensor("wextb", [P, WCOLS], bf16, kind="ExternalInput")
    ownxt = nc.dram_tensor("ownxt", [P, slots * P], bf16,
                           kind="ExternalInput")
    gidx_d = nc.dram_tensor("gidx", [P, gc16], mybir.dt.int16,
                            kind="ExternalInput")
    biasb = nc.dram_tensor("biasb", [P, F_OUT], fp32, kind="ExternalInput")
    padrow = nc.dram_tensor("padrow", [1, P], bf16, kind="ExternalInput")
    out_d = nc.dram_tensor("out", [slots * P, F_OUT], fp32,
                           kind="ExternalOutput")
    tbl = nc.dram_tensor("tbl", [TROWS, P], bf16, kind="Internal")

    with tile.TileContext(nc) as tc:
        with (
            tc.tile_pool(name="const", bufs=1) as cpool,
            tc.tile_pool(name="xt", bufs=3) as xtpool,
            tc.tile_pool(name="ps", bufs=4, space="PSUM") as pspool,
            tc.tile_pool(name="tstage", bufs=3) as tspool,
            tc.tile_pool(name="gat", bufs=5) as gpool,
            tc.tile_pool(name="wgt", bufs=3) as wpool,
            tc.tile_pool(name="sc", bufs=3) as scpool,
            tc.tile_pool(name="blk", bufs=3) as bpool,
        ):
            wextb_sb = cpool.tile([P, WCOLS], bf16)
            nc.sync.dma_start(out=wextb_sb[:], in_=wextb[:])
            biasb_sb = cpool.tile([P, F_OUT], fp32)
            nc.sync.dma_start(out=biasb_sb[:], in_=biasb[:])
            ownxt_sb = cpool.tile([P, slots * P], bf16)
            nc.sync.dma_start(out=ownxt_sb[:], in_=ownxt[:])
            gidx_sb = cpool.tile([P, gc16], mybir.dt.int16)
            nc.sync.dma_start(out=gidx_sb[:], in_=gidx_d[:])

            # ---- own-node precompute ----
            h_own = cpool.tile([P, slots, F_OUT], bf16)
            aos = cpool.tile([P, slots], fp32)
            aod = cpool.tile([P, slots], fp32)
            for i in range(slots):
                ps2 = pspool.tile([P, WCOLS], fp32, tag="own")
                nc.tensor.matmul(out=ps2[:], lhsT=ownxt_sb[:, i * P:(i + 1) * P],
                                 rhs=wextb_sb[:], start=True, stop=True)
                if i % 2 == 0:
                    nc.scalar.copy(out=h_own[:, i, :], in_=ps2[:, 0:F_OUT])
                else:
                    nc.vector.tensor_copy(out=h_own[:, i, :],
                                          in_=ps2[:, 0:F_OUT])
                nc.vector.tensor_copy(out=aos[:, i:i + 1],
                                      in_=ps2[:, F_OUT:F_OUT + 1])
                nc.vector.tensor_copy(out=aod[:, i:i + 1],
                                      in_=ps2[:, F_OUT + 1:F_OUT + 2])
            zown = cpool.tile([P, slots], fp32)
            nc.vector.tensor_tensor(out=zown[:], in0=aos[:], in1=aod[:],
                                    op=mybir.AluOpType.add)
            e1o = cpool.tile([P, slots], fp32)
            e2o = cpool.tile([P, slots], fp32)
            nc.scalar.activation(out=e1o[:], in_=zown[:],
                                 func=mybir.ActivationFunctionType.Exp,
                                 scale=1.0)
            nc.scalar.activation(out=e2o[:], in_=zown[:],
                                 func=mybir.ActivationFunctionType.Exp,
                                 scale=NEG_SLOPE)
            s_ii = cpool.tile([P, slots], fp32)
            nc.vector.tensor_tensor(out=s_ii[:], in0=e1o[:], in1=e2o[:],
                                    op=mybir.AluOpType.max)
            s_ii16 = cpool.tile([P, slots], bf16)
            nc.vector.tensor_copy(out=s_ii16[:], in_=s_ii[:])

            # ---- phase A: table build (first-use row order) ----
            WB = 8
            nblk_tbl = TROWS // P
            for g0 in range(0, nblk_tbl, WB):
                gn = min(WB, nblk_tbl - g0)
                xtb8 = xtpool.tile([P, WB, P], bf16, tag="xtb")
                nc.sync.dma_start(
                    out=xtb8[:, 0:gn, :],
                    in_=xTb[:, g0 * P:(g0 + gn) * P].rearrange(
                        "p (i q) -> p i q", q=P))
                tstage = tspool.tile([P, WB, 68], bf16)
                for h0 in range(0, gn, 4):
                    hn = min(4, gn - h0)
                    ps4 = pspool.tile([P, 4, WCOLS], fp32, tag="tbl")
                    for bi in range(hn):
                        nc.tensor.matmul(out=ps4[:, bi, :],
                                         lhsT=xtb8[:, h0 + bi, :].squeeze(),
                                         rhs=wextb_sb[:],
                                         start=True, stop=True)
                    if h0 == 0:
                        nc.scalar.copy(out=tstage[:, h0:h0 + hn, 0:F_OUT],
                                       in_=ps4[:, 0:hn, 0:F_OUT])
                    else:
                        nc.vector.tensor_copy(out=tstage[:, h0:h0 + hn, 0:F_OUT],
                                              in_=ps4[:, 0:hn, 0:F_OUT])
                    nc.vector.tensor_copy(
                        out=tstage[:, h0:h0 + hn, F_OUT:F_OUT + 4].bitcast(fp32),
                        in_=ps4[:, 0:hn, F_OUT:F_OUT + 2])
                r0, r1 = g0 * P, (g0 + gn) * P
                nc.scalar.dma_start(
                    out=tbl[r0:r1, 0:68].rearrange("(i p) w -> p i w", p=P),
                    in_=tstage[:, 0:gn, :])
                if r0 == 0:
                    nc.sync.dma_start(out=tbl[0:1, :], in_=padrow[:])
                if r0 <= MID < r1:
                    nc.sync.dma_start(out=tbl[MID:MID + 1, :], in_=padrow[:])

            # ---- phase B ----
            calls_by_sc = {}
            for cl in calls:
                calls_by_sc.setdefault(cl["ci"], []).append(cl)

            for ci, sc in enumerate(scs):
                offA, offB = sc_offs[ci]
                cA, cB = int(offA[-1]), int(offB[-1])
                ncols = cA + cB
                nb = len(sc)
                i0 = sc[0]
                g_t = gpool.tile([P, ncols, P], bf16)
                for cl in calls_by_sc[ci]:
                    base = cl["c0"] + (cA if cl["side"] else 0)
                    n_i = (cl["c1"] - cl["c0"]) * P
                    in_ap = (tbl[0:cl["pref"], :] if cl["side"] == 0
                             else tbl[MID:cl["pref"], :])
                    nc.gpsimd.dma_gather(
                        out_ap=g_t[:, base:base + (cl["c1"] - cl["c0"]), :],
                        in_ap=in_ap,
                        idxs_ap=gidx_sb[:, cl["off16"]:cl["off16"] + cl["ln16"]],
                        num_idxs=n_i, num_idxs_reg=n_i,
                        elem_size=P, single_packet=False, queue_num=cl["q"])

                # adst per column (broadcast per slot-range), then z, exps
                az = scpool.tile([P, ncols], fp32, tag="az")
                for bi, s in enumerate(sc):
                    for (c0, c1) in ((int(offA[bi]), int(offA[bi + 1])),
                                     (cA + int(offB[bi]), cA + int(offB[bi + 1]))):
                        if c1 > c0:
                            nc.vector.tensor_copy(
                                out=az[:, c0:c1],
                                in_=aod[:, s:s + 1].broadcast_to([P, c1 - c0]))
                asrc_v = g_t[:, :, F_OUT:F_OUT + 2].bitcast(fp32)
                nc.vector.tensor_tensor(out=az[:], in0=az[:],
                                        in1=asrc_v.squeeze(),
                                        op=mybir.AluOpType.add)
                e1 = scpool.tile([P, ncols], fp32, tag="e1")
                e2 = scpool.tile([P, ncols], fp32, tag="e2")
                nc.scalar.activation(out=e1[:], in_=az[:],
                                     func=mybir.ActivationFunctionType.Exp,
                                     scale=1.0)
                nc.scalar.activation(out=e2[:], in_=az[:],
                                     func=mybir.ActivationFunctionType.Exp,
                                     scale=NEG_SLOPE)
                s_t = scpool.tile([P, ncols], fp32, tag="s")
                nc.vector.tensor_tensor(out=s_t[:], in0=e1[:], in1=e2[:],
                                        op=mybir.AluOpType.max)
                s16 = scpool.tile([P, ncols], bf16, tag="s16")
                nc.scalar.copy(out=s16[:], in_=s_t[:])
                wgt = wpool.tile([P, ncols, F_OUT], bf16)
                nc.vector.tensor_tensor(
                    out=wgt[:], in0=g_t[:, :, 0:F_OUT],
                    in1=s16[:].unsqueeze(2).broadcast_to([P, ncols, F_OUT]),
                    op=mybir.AluOpType.mult)

                dn = bpool.tile([P, 2 * nb], fp32, tag="dn")
                wsA = bpool.tile([P, nb, F_OUT], fp32, tag="wsA")
                wsB = bpool.tile([P, nb, F_OUT], fp32, tag="wsB")
                for bi, s in enumerate(sc):
                    for hi, (ws, (c0, c1)) in enumerate(
                        ((wsA, (int(offA[bi]), int(offA[bi + 1]))),
                         (wsB, (cA + int(offB[bi]), cA + int(offB[bi + 1]))))):
                        j = 2 * bi + hi
                        if c1 > c0:
                            nc.vector.tensor_reduce(
                                out=dn[:, j:j + 1], in_=s_t[:, c0:c1],
                                axis=mybir.AxisListType.X,
                                op=mybir.AluOpType.add)
                            nc.vector.tensor_reduce(
                                out=ws[:, bi, :],
                                in_=wgt[:, c0:c1, :].rearrange(
                                    "p c f -> p f c"),
                                axis=mybir.AxisListType.X,
                                op=mybir.AluOpType.add)
                        else:
                            nc.vector.memset(dn[:, j:j + 1], 0.0)
                            nc.vector.memset(ws[:, bi, :], 0.0)

                den = bpool.tile([P, nb], fp32, tag="den")
                nc.vector.tensor_reduce(
                    out=den[:], in_=dn[:].rearrange("p (b t) -> p b t", t=2),
                    axis=mybir.AxisListType.X, op=mybir.AluOpType.add)
                nc.vector.tensor_add(den[:], den[:], s_ii[:, i0:i0 + nb])
                rec = bpool.tile([P, nb], fp32, tag="rec")
                nc.vector.reciprocal(rec[:], den[:])

                num = bpool.tile([P, nb, F_OUT], fp32, tag="num")
                nc.vector.tensor_add(num[:], wsA[:], wsB[:])
                selfm = bpool.tile([P, nb, F_OUT], fp32, tag="selfm")
                nc.vector.tensor_tensor(
                    out=selfm[:], in0=h_own[:, i0:i0 + nb, :],
                    in1=s_ii16[:, i0:i0 + nb].unsqueeze(2).broadcast_to(
                        [P, nb, F_OUT]),
                    op=mybir.AluOpType.mult)
                nc.vector.tensor_add(num[:], num[:], selfm[:])
                nc.vector.tensor_tensor(
                    out=num[:], in0=num[:],
                    in1=rec[:].unsqueeze(2).broadcast_to([P, nb, F_OUT]),
                    op=mybir.AluOpType.mult)
                nc.vector.tensor_tensor(
                    out=num[:], in0=num[:],
                    in1=biasb_sb[:].unsqueeze(1).broadcast_to([P, nb, F_OUT]),
                    op=mybir.AluOpType.add)
                ostage = bpool.tile([P, nb, F_OUT], fp32, tag="ostage")
                nc.scalar.activation(out=ostage[:], in_=num[:],
                                     func=mybir.ActivationFunctionType.Relu)
                nc.sync.dma_start(
                    out=out_d[i0 * P:(i0 + nb) * P, :].rearrange(
                        "(i p) f -> p i f", p=P),
                    in_=ostage[:])
    nc.compile()
    return nc


# --------------------------------------------------------------------------
# entry point
# --------------------------------------------------------------------------

_RUN_KW = {}
_LAST_RESULT = [None]


def kernel(x, edge_index, W, att_src, att_dst, bias):
    x = np.asarray(x, dtype=np.float32)
    W = np.asarray(W, dtype=np.float32)
    att_src = np.asarray(att_src, dtype=np.float32)
    att_dst = np.asarray(att_dst, dtype=np.float32)
    bias = np.asarray(bias, dtype=np.float32)
    n_nodes = x.shape[0]

    meta = _preprocess(edge_index, n_nodes)
    lists, sc_offs = _build_gather_lists(meta)
    calls, gidx, gc16 = _plan_calls(meta, lists)

    cfg = dict(slots=meta["slots"], scs=meta["scs"], sc_offs=sc_offs,
               D_A=meta["D_A"], D_B=meta["D_B"], calls=calls, gc16=gc16)
    nc = _build_nc(cfg)

    wext = np.zeros((P, WCOLS), np.float32)
    wext[:, 0:F_OUT] = W
    wext[:, F_OUT] = W @ att_src
    wext[:, F_OUT + 1] = W @ att_dst
    wextb = wext.astype(ml_dtypes.bfloat16)

    xT = np.zeros((P, TROWS), np.float32)
    m = meta["rank"] >= 0
    xT[:, meta["pos_t"][m]] = x[m].T
    xTb = xT.astype(ml_dtypes.bfloat16)

    biasb_h = np.tile(bias[None, :], (P, 1)).astype(np.float32)
    padrow_f32 = np.zeros(P // 2, dtype=np.float32)
    padrow_f32[F_OUT // 2] = PAD_ASRC
    padrow_h = padrow_f32.view(ml_dtypes.bfloat16).reshape(1, P).copy()

    gmax = max(gc16, 16)
    in_maps = []
    for k in range(NCORES):
        ox = np.zeros((P, meta["slots"] * P), np.float32)
        nd = meta["node_at"][k].reshape(-1)
        mv = nd >= 0
        ox[:, mv] = x[nd[mv]].T
        gi = gidx[k]
        if gi.shape[1] < gmax:
            gi = np.concatenate(
                [gi, np.zeros((P, gmax - gi.shape[1]), np.int16)], axis=1)
        in_maps.append({
            "xTb": xTb, "wextb": wextb,
            "ownxt": ox.astype(ml_dtypes.bfloat16),
            "gidx": np.ascontiguousarray(gi),
            "biasb": biasb_h,
            "padrow": padrow_h,
        })

    res = run_bass_kernel_spmd(nc, in_maps, core_ids=list(range(NCORES)),
                               **_RUN_KW)
    _LAST_RESULT[0] = res

    out = np.zeros((n_nodes, F_OUT), dtype=np.float32)
    for k in range(NCORES):
        nd = meta["node_at"][k].reshape(-1)
        mv = nd >= 0
        out[nd[mv]] = res.results[k]["out"][mv]
    return out
